# revision 27
# baseline (speedup 1.0000x reference)
"""Trainium2 Bass kernel for nn_Code_Wise_Attention.

Reference computation (per batch element b):
    S[c, q]   = context[c, :] . query[q, :]          # [Lc, Lq]
    scores[c] = max_q S[c, q]                        # [Lc]
    attn      = softmax(scores)                      # [Lc]
    out[d]    = sum_c attn[c] * context[c, d]        # [1, D]

Sharding: data-parallel over batch B=8 across the 8 NeuronCores; each core
handles one batch element end-to-end, no cross-core communication.

Per-core design (PE-bound; the S matmul floor is ~55us, everything else is
arranged to hide under it):
  - Load Q [2048,256] and C [4096,256] into SBUF as float32r views (chunked
    DMAs, Q first: the band consumes all of qT from its first tile).
  - PE-transpose 128x128 blocks into d-major layouts (the matmul contracts
    over the partition axis). float32r transposes run at 1.5 cyc/row and
    are batched: several blocks land in one PSUM bank and one wide
    VectorE/ScalarE copy moves them out. Q-transpose pairs are interleaved
    with the first context tile's matmuls so the band starts as soon as
    the first DMA chunks land; C transposes run 3 tiles ahead of the band.
  - S matmul in float32r (fp32 operands truncated to FP22; 1 cycle/row at
    moving free dim >= 256 -- bf16 speed at ~2^-13 relative precision).
    Per context tile: 8 matmuls into two [128,1024] PSUM tiles, ordered
    h-outer so same-bank accumulation steps are 4 matmuls apart (never
    stalls on the PE->PSUM write latency).
  - Row max over q (the only heavy reduction, 8.4M values): VectorE
    tensor_scalar(max, max-accum) directly on one PSUM tile, ScalarE
    copies the other to SBUF where a second tensor_scalar pass (2x-mode
    eligible) finishes the row, seeded with the first pass's partial.
  - Softmax without a global pass: after tile 0, its all-reduced row max
    becomes a provisional max M^; every tile's exp(score - M^) is computed
    in-band on ScalarE and the unnormalized weighted context sum
    accumulates on the otherwise-idle GPSIMD (tensor_tensor mult with a
    broadcast column + add). M^ cancels in the final normalization, and
    exp arguments are bounded by the score spread (<< fp32 overflow).
  - Finalize: two 1-wide fp32 matmuls with a ones vector partition-sum the
    accumulator and the exp table, then reciprocal, scale, DMA out.
"""

import sys

import numpy as np

if "/opt/trn_rl_repo" not in sys.path:
    sys.path.insert(0, "/opt/trn_rl_repo")

B = 8
LQ = 2048
LC = 4096
D = 256
P = 128
N_CORES = 8

_CACHE = {}


def _build_nc():
    import concourse.bass as bass  # noqa: F401
    import concourse.tile as tile
    from concourse import bacc, bass_isa, mybir
    from concourse.bass import ts
    from concourse.masks import make_identity

    f32 = mybir.dt.float32
    f32r = mybir.dt.float32r
    AL = mybir.AluOpType
    AF = mybir.ActivationFunctionType

    Q_TILES = LQ // P  # 16
    C_TILES = LC // P  # 32
    KH = D // P  # 2 k-halves of the 256-deep contraction

    nc = bacc.Bacc("TRN2", target_bir_lowering=False, debug=False)

    q_dram = nc.dram_tensor("query", [LQ, D], f32, kind="ExternalInput").ap()
    c_dram = nc.dram_tensor("context", [LC, D], f32, kind="ExternalInput").ap()
    o_dram = nc.dram_tensor("out", [1, D], f32, kind="ExternalOutput").ap()

    q_tiled = q_dram.rearrange("(t p) d -> p t d", p=P)  # [128, 16, 256]
    c_tiled = c_dram.rearrange("(t p) d -> p t d", p=P)  # [128, 32, 256]

    with tile.TileContext(nc) as tc:
        with (
            tc.tile_pool(name="singles", bufs=1) as singles,
            tc.tile_pool(name="scopy", bufs=2) as scopy_pool,
            tc.tile_pool(name="trash", bufs=2) as trash_pool,
            tc.tile_pool(name="gpool", bufs=2) as gpool,
        ):
            ident_f32 = singles.tile([P, P], f32)
            make_identity(nc, ident_f32)
            ident = singles.tile([P, P], f32r)
            nc.vector.tensor_copy(out=ident, in_=ident_f32)

            # Warm the ACT exp table set early so the ~2.7us table load
            # overlaps the input DMAs instead of the first in-band exp.
            junk_in = singles.tile([P, 1], f32)
            nc.vector.memset(junk_in, 0.0)
            junk_out = singles.tile([P, 1], f32)
            nc.scalar.activation(junk_out, junk_in, AF.Exp)

            q_sb = singles.tile([P, Q_TILES, D], f32r)
            c_sb = singles.tile([P, C_TILES, D], f32r)
            qT = singles.tile([P, KH, LQ], f32r)
            cT = singles.tile([P, KH, LC], f32r)
            scores = singles.tile([P, C_TILES], f32)
            partials = singles.tile([P, C_TILES], f32)

            # Chunked input DMAs so transposes can start before the full
            # tensors land. All of Q first (the band needs full qT), then C.
            for g in range(8):
                nc.sync.dma_start(
                    q_sb[:, g * 2 : (g + 1) * 2, :],
                    q_tiled[:, g * 2 : (g + 1) * 2, :].bitcast(f32r),
                )
            for g in range(8):
                nc.sync.dma_start(
                    c_sb[:, g * 4 : (g + 1) * 4, :],
                    c_tiled[:, g * 4 : (g + 1) * 4, :].bitcast(f32r),
                )

            # ---- transpose helpers
            # qT[dh, h, t*128 + p] = Q[t*128 + p, h*128 + dh]: pairs of
            # q-tiles transpose into one PSUM bank laid out [h, t, 128];
            # a single wide copy moves all four blocks.
            qtpsum_cm = tc.tile_pool(name="qtpsum", bufs=2, space="PSUM")
            qtpsum = qtpsum_cm.__enter__()

            def qt_pair(tp2):
                t0 = 2 * tp2
                qp = qtpsum.tile([P, 2, 2, P], f32r, tag="qp", name=f"qp{tp2}")
                for h in range(KH):
                    for dt_ in range(2):
                        nc.tensor.transpose(
                            qp[:, h, dt_, :],
                            q_sb[:, t0 + dt_, ts(h, P)],
                            ident,
                        )
                dst = qT[:, :, t0 * P : (t0 + 2) * P].rearrange(
                    "p h (t f) -> p h t f", t=2
                )
                if tp2 % 2 == 0:
                    nc.vector.tensor_copy(out=dst, in_=qp[:, :, :, :])
                else:
                    nc.scalar.copy(dst, qp[:, :, :, :])

            def c_transpose(t, pool):
                tp = pool.tile([P, KH, P], f32r, tag="tp", name=f"ctp_{t}")
                for h in range(KH):
                    nc.tensor.transpose(
                        tp[:, h, :], c_sb[:, t, ts(h, P)], ident
                    )
                nc.scalar.copy(cT[:, :, ts(t, P)], tp[:, :, :])

            qt_pair(0)
            qt_pair(1)
            c_transpose(0, qtpsum)
            c_transpose(1, qtpsum)

            exp_s = singles.tile([P, C_TILES], f32)
            neg_mhat = singles.tile([P, 1], f32)
            acc = singles.tile([P, D], f32)
            nc.gpsimd.memset(acc, 0.0)

            def exp_and_accum(t, on_dve=False):
                # Unnormalized weight relative to the provisional max
                # (tile 0's row max, all-reduced) -- cancels in the final
                # normalization, so no global max pass is needed.
                nc.scalar.activation(
                    exp_s[:, t : t + 1],
                    scores[:, t : t + 1],
                    AF.Exp,
                    bias=neg_mhat[:, :],
                    scale=1.0,
                )
                if on_dve:
                    # Tail only: the band is over, VectorE is free.
                    nc.vector.scalar_tensor_tensor(
                        out=acc,
                        in0=c_sb[:, t, :].bitcast(f32),
                        scalar=exp_s[:, t : t + 1],
                        in1=acc,
                        op0=AL.mult,
                        op1=AL.add,
                    )
                else:
                    prod = gpool.tile([P, D], f32)
                    nc.gpsimd.tensor_tensor(
                        out=prod,
                        in0=c_sb[:, t, :].bitcast(f32),
                        in1=exp_s[:, t : t + 1].to_broadcast((P, D)),
                        op=AL.mult,
                    )
                    nc.gpsimd.tensor_tensor(
                        out=acc, in0=acc, in1=prod, op=AL.add
                    )

            # ---- main band over context tiles
            tpsum = None
            spsum = None
            psum_stack = None
            spsum_cm = None
            for ct in range(C_TILES):
                if ct == 0:
                    psA = qtpsum.tile([P, 1024], f32, tag="s", name="psA_0")
                    psB = qtpsum.tile([P, 1024], f32, tag="s", name="psB_0")
                    # qc inner so each chunk's matmuls follow its
                    # q-transposes as the DMA chunks land.
                    for qc in range(4):
                        if qc >= 1:
                            qt_pair(2 * qc)
                            qt_pair(2 * qc + 1)
                        pst = psA if qc < 2 else psB
                        dst = pst[:, (qc % 2) * 512 : (qc % 2 + 1) * 512]
                        for h in range(KH):
                            nc.tensor.matmul(
                                dst,
                                lhsT=cT[:, h, ts(ct, P)],
                                rhs=qT[:, h, ts(qc, 512)],
                                start=(h == 0),
                                stop=(h == KH - 1),
                            )
                    c_transpose(2, qtpsum)
                    c_transpose(3, qtpsum)
                elif ct < C_TILES - 1:
                    psA = spsum.tile([P, 1024], f32, tag="s", name=f"psA_{ct}")
                    psB = spsum.tile([P, 1024], f32, tag="s", name=f"psB_{ct}")
                    # h outer: same-bank accumulation steps are 4 matmuls
                    # apart, so the PE never waits on its PSUM write latency.
                    for h in range(KH):
                        for qc in range(4):
                            pst = psA if qc < 2 else psB
                            dst = pst[:, (qc % 2) * 512 : (qc % 2 + 1) * 512]
                            nc.tensor.matmul(
                                dst,
                                lhsT=cT[:, h, ts(ct, P)],
                                rhs=qT[:, h, ts(qc, 512)],
                                start=(h == 0),
                                stop=(h == KH - 1),
                            )
                else:
                    # Last tile: finish chunks as early as possible and
                    # reduce each straight from PSUM so the epilogue chain
                    # after the final matmul is short.
                    psA = spsum.tile([P, 1024], f32, tag="s", name=f"psA_{ct}")
                    psB = spsum.tile([P, 1024], f32, tag="s", name=f"psB_{ct}")
                    for qp_ in range(2):
                        pst = psA if qp_ == 0 else psB
                        for h in range(KH):
                            for qc2 in range(2):
                                nc.tensor.matmul(
                                    pst[:, qc2 * 512 : (qc2 + 1) * 512],
                                    lhsT=cT[:, h, ts(ct, P)],
                                    rhs=qT[:, h, ts(2 * qp_ + qc2, 512)],
                                    start=(h == 0),
                                    stop=(h == KH - 1),
                                )

                if ct < C_TILES - 1:
                    # psA staged to SBUF by ACT; psB reduced straight from
                    # PSUM on DVE; then the SBUF copy is reduced (2x-mode
                    # eligible) seeded with the psB partial.
                    sc = scopy_pool.tile([P, 1024], f32)
                    nc.scalar.copy(sc, psA)
                    trB = trash_pool.tile([P, 1024], f32, tag="t0")
                    nc.vector.tensor_scalar(
                        out=trB,
                        in0=psB,
                        scalar1=-3.0e38,
                        scalar2=None,
                        op0=AL.max,
                        op1=AL.max,
                        accum_out=partials[:, ct : ct + 1],
                    )
                    trA = trash_pool.tile([P, 1024], f32, tag="t1")
                    nc.vector.tensor_scalar(
                        out=trA,
                        in0=sc,
                        scalar1=partials[:, ct : ct + 1],
                        scalar2=None,
                        op0=AL.max,
                        op1=AL.max,
                        accum_out=scores[:, ct : ct + 1],
                    )
                else:
                    for j, pst in enumerate((psA, psB)):
                        trB = trash_pool.tile(
                            [P, 1024], f32, tag="t0", name=f"trl{j}"
                        )
                        nc.vector.tensor_scalar(
                            out=trB,
                            in0=pst,
                            scalar1=(
                                -3.0e38 if j == 0
                                else partials[:, ct : ct + 1]
                            ),
                            scalar2=None,
                            op0=AL.max,
                            op1=AL.max,
                            accum_out=(
                                partials[:, ct : ct + 1] if j == 0
                                else scores[:, ct : ct + 1]
                            ),
                        )

                if ct == 0:
                    qtpsum_cm.__exit__(None, None, None)
                    psum_stack = tc.tile_pool(name="tpsum", bufs=2, space="PSUM")
                    tpsum = psum_stack.__enter__()
                    spsum_cm = tc.tile_pool(name="spsum", bufs=3, space="PSUM")
                    spsum = spsum_cm.__enter__()
                elif ct + 3 < C_TILES:
                    c_transpose(ct + 3, tpsum)

                if ct == 0:
                    mhat = singles.tile([P, 1], f32)
                    nc.gpsimd.partition_all_reduce(
                        mhat,
                        scores[:, 0:1],
                        channels=P,
                        reduce_op=bass_isa.ReduceOp.max,
                    )
                    nc.vector.tensor_scalar_mul(neg_mhat, mhat, -1.0)
                if ct >= 1:
                    exp_and_accum(ct - 1)

            exp_and_accum(C_TILES - 1, on_dve=True)

            spsum_cm.__exit__(None, None, None)
            psum_stack.__exit__(None, None, None)

            # ---- finalize: partition-sum exp_s (-> total) and acc with
            # 1-wide ones matmuls, then normalize and write out.
            ones = singles.tile([P, 1], f32)
            nc.vector.memset(ones, 1.0)
            with tc.tile_pool(name="wpsum", bufs=2, space="PSUM") as wpsum:
                po2 = wpsum.tile([1, C_TILES], f32)
                nc.tensor.matmul(
                    po2, lhsT=ones, rhs=exp_s, start=True, stop=True
                )
                tot = singles.tile([1, 1], f32)
                trs = trash_pool.tile([1, C_TILES], f32, tag="t0")
                nc.vector.tensor_scalar(
                    out=trs,
                    in0=po2,
                    scalar1=0.0,
                    scalar2=None,
                    op0=AL.add,
                    op1=AL.add,
                    accum_out=tot,
                )
                recip = singles.tile([1, 1], f32)
                nc.vector.reciprocal(recip, tot)
                po = wpsum.tile([1, D], f32)
                nc.tensor.matmul(po, lhsT=ones, rhs=acc, start=True, stop=True)
                out_sb = singles.tile([1, D], f32)
                nc.vector.tensor_scalar_mul(out_sb, po, recip[0:1, :])
                nc.sync.dma_start(o_dram, out_sb)

    nc.compile()
    return nc


def _get_nc():
    if "nc" not in _CACHE:
        _CACHE["nc"] = _build_nc()
    return _CACHE["nc"]


def run(query, context, trace=False):
    """Run on 8 cores; returns (out [8,1,256], BassKernelResults)."""
    from concourse.bass_utils import run_bass_kernel_spmd

    nc = _get_nc()
    query = np.asarray(query, dtype=np.float32)
    context = np.asarray(context, dtype=np.float32)
    assert query.shape == (B, LQ, D), query.shape
    assert context.shape == (B, LC, D), context.shape

    in_maps = [
        {
            "query": np.ascontiguousarray(query[b]),
            "context": np.ascontiguousarray(context[b]),
        }
        for b in range(N_CORES)
    ]
    try:
        res = run_bass_kernel_spmd(
            nc, in_maps, core_ids=list(range(N_CORES)), trace=trace
        )
    except Exception:
        # One retry: a crashed prior process can leave a core wedged; the
        # runtime usually recovers it on the next attempt.
        res = run_bass_kernel_spmd(
            nc, in_maps, core_ids=list(range(N_CORES)), trace=trace
        )
    out = np.stack([res.results[b]["out"] for b in range(N_CORES)], axis=0)
    return out.reshape(B, 1, D), res


def kernel(query, context):
    out, _ = run(query, context, trace=False)
    return out
